# revision 45
# baseline (speedup 1.0000x reference)
"""Multi-head causal self-attention on 8 Trainium2 NeuronCores.

Problem: X[4,2048,1024], per-head Wq/Wk/Wv[16,1024,64], Wo[1024,1024], bo[1024].
    out = OutProj(concat_heads(softmax_causal(Q K^T / 8) V))

Sharding: 8 cores = 4 batches x 2 head-groups (8 heads each). Each core
computes its batch's attention for its 8 heads plus the partial output
projection over its 512 concat features; host sums the two partials per
batch and adds the bias.

Per-core kernel (matmul operands in fp16 — 1 cycle/row on TensorE and
fp32 PSUM accumulation; softmax runs in the transposed
"feature-on-partition" space so its reduction lands on the free dim):
  qT/kT per head-pair  [128, T]  = Wpair^T  x  X^T
  v    per s-tile      [128, 8*65] = X^T^T  x  Wv_all (65th col set to 1)
  ST block [s=128, t=512] = kT_slice^T @ qT_slice   (row-packed head pairs)
  expST = exp(ST/8) (ScalarE), causal-masked via tri multiply
  avT [65, 512] += [V|1]^T @ expST   -> rows 0:64 = (A@V)^T, row 64 = sums
  normalize via 1/sums broadcast and write concatT
  partial = concatT^T @ WoST  (accumulated over 4 feature chunks)

Scheduling: everything is emitted in DMA-arrival order. X streams in
T-block order; pair 0's tt>0 projections are deferred into its attention
loop (need-forced per query tile) so the first scores fire as soon as
the first X block lands. Warmup matmuls fill the DMA window and keep the
PE HAM clock at 8/8. The output projection is interleaved into pair 3's
attention as PE filler; pair 3 normalizes in 128-column slices so each
out-proj group unblocks as early as possible.
"""

import os
import sys

for _p in ("/opt/trn_rl_repo", "/root/.axon_site/_ro/trn_rl_repo"):
    if os.path.isdir(_p) and _p not in sys.path:
        sys.path.append(_p)

import numpy as np

import concourse.mybir as mybir
import concourse.tile as tile
from concourse import bacc

B, T, D, H, K = 4, 2048, 1024, 16, 64
HG = 8          # heads per core
NPAIR = 4       # head pairs per core
P = 128
DC = D // P     # 8 contraction chunks for the projections
NS = T // P     # 16 key tiles
NT = T // 512   # 4 query tiles of 512
F32 = mybir.dt.float32
F16 = mybir.dt.float16


def build_module():
    nc = bacc.Bacc("TRN2")
    XT = nc.dram_tensor("xt", [D, T], F16, kind="ExternalInput").ap()
    WQ = nc.dram_tensor("wq", [NPAIR, D, P], F16, kind="ExternalInput").ap()
    WK = nc.dram_tensor("wk", [NPAIR, D, P], F16, kind="ExternalInput").ap()
    WV = nc.dram_tensor("wv", [D, HG * K], F16, kind="ExternalInput").ap()
    WO = nc.dram_tensor("wo", [HG * K, D], F16, kind="ExternalInput").ap()
    OUT = nc.dram_tensor("out", [T, D], F16, kind="ExternalOutput").ap()

    with tile.TileContext(nc) as tc:
        with tc.tile_pool(name="persist", bufs=1) as pp:
            xt_sb = pp.tile([P, DC, T], F16)            # X^T, 32 KB/partition
            v_sb = pp.tile([P, NS, HG * (K + 1)], F16)  # V + ones col per head
            concat_sb = pp.tile([P, NPAIR, T], F16)     # concat(heads)^T
            tri_sb = pp.tile([P, P], F16)   # causal triangle: 1 where x >= p
            warm_sb = pp.tile([P, 512], F16)

            # X^T in T-block order: the first 512 token-cols of every chunk
            # land first so V/Q/K tile 0 start as soon as possible. Each
            # DMA queue only sustains ~110-220 GB/s, so the critical
            # startup set (first X block + Wv + pair-0 Wq/Wk) is spread
            # over all three trigger queues; the X tail and Wo follow in
            # arrival-deadline order.
            xt_r = XT.rearrange("(c p) t -> c p t", p=P)
            wo_sb = pp.tile([P, NPAIR, D], F16)
            for c in range(DC):
                nc.sync.dma_start(out=xt_sb[:, c, 0:512], in_=xt_r[c][:, 0:512])
            nc.vector.memset(warm_sb, 0.0)
            nc.vector.memset(tri_sb, 1.0)
            nc.gpsimd.affine_select(
                out=tri_sb,
                in_=tri_sb,
                compare_op=mybir.AluOpType.is_ge,
                fill=0.0,
                base=0,
                channel_multiplier=-1,
                pattern=[[1, P]],
            )
            # ones column (index 64 of each head's 65-wide slot)
            v_slots = v_sb.rearrange("p s (h x) -> p s h x", x=K + 1)
            nc.vector.memset(v_slots[:, :, :, K : K + 1], 1.0)

            # ---- V + Q/K projections + attention, software-pipelined ----
            # Projection matmuls (V tail, pair-0 tt>0, the NEXT pair's Q/K)
            # and pair-3's output projection are interleaved into the
            # attention loop so the PE fills the stalls where it would
            # otherwise wait on ScalarE's exp.
            with (
                tc.tile_pool(name="wvp", bufs=1) as wvp,
                tc.tile_pool(name="attn", bufs=1) as ap_,
            ):
                psa = None  # assigned after the startup PSUM pool closes
                wv_sb = wvp.tile([P, DC, HG * K], F16)
                wv_r = WV.rearrange("(c p) n -> c p n", p=P)
                for c in range(DC):
                    (nc.scalar if c < 4 else nc.gpsimd).dma_start(
                        out=wv_sb[:, c, :], in_=wv_r[c]
                    )

                def v_group_ops(s, pool=None, bufs=2):
                    ps = (pool or psa).tile(
                        [P, HG * K], F32, tag="mm", bufs=bufs, name=f"vps{s}"
                    )
                    ops = [
                        (
                            lambda c=c, ps=ps, s=s: nc.tensor.matmul(
                                ps,
                                xt_sb[:, c, s * P : (s + 1) * P],
                                wv_sb[:, c, :],
                                start=(c == 0),
                                stop=(c == DC - 1),
                            )
                        )
                        for c in range(DC)
                    ]
                    ops.append(
                        lambda ps=ps, s=s: nc.vector.tensor_copy(
                            v_slots[:, s, :, 0:K],
                            ps.rearrange("p (h k) -> p h k", k=K),
                        )
                    )
                    return ops

                def proj_weights(pr, split=False):
                    """DMA the pair's Wq/Wk; alloc the q/k destinations."""
                    wq_sb = ap_.tile(
                        [P, DC, P], F16, tag="wq", bufs=2, name=f"wq{pr}"
                    )
                    wk_sb = ap_.tile(
                        [P, DC, P], F16, tag="wk", bufs=2, name=f"wk{pr}"
                    )
                    nc.scalar.dma_start(
                        out=wq_sb, in_=WQ[pr].rearrange("(c p) m -> p c m", p=P)
                    )
                    (nc.gpsimd if split else nc.scalar).dma_start(
                        out=wk_sb, in_=WK[pr].rearrange("(c p) m -> p c m", p=P)
                    )
                    q_sb = ap_.tile([P, T], F16, tag="q", bufs=2, name=f"q{pr}")
                    k_sb = ap_.tile([P, T], F16, tag="k", bufs=2, name=f"k{pr}")
                    return wq_sb, wk_sb, q_sb, k_sb

                def proj_ops(pr, tiles, tts, pool=None, bufs=2):
                    """Projection matmuls in tt-major (DMA-arrival) order."""
                    wq_sb, wk_sb, q_sb, k_sb = tiles
                    ops = []
                    for tt in tts:
                        for w_sb, qk_sb, nm in (
                            (wq_sb, q_sb, "q"),
                            (wk_sb, k_sb, "k"),
                        ):
                            ps = (pool or psa).tile(
                                [P, 512], F32, tag="mm", bufs=bufs,
                                name=f"{nm}ps{pr}_{tt}",
                            )
                            for c in range(DC):
                                ops.append(
                                    lambda ps=ps, w_sb=w_sb, c=c, tt=tt:
                                    nc.tensor.matmul(
                                        ps,
                                        w_sb[:, c, :],
                                        xt_sb[
                                            :, c, tt * 512 : (tt + 1) * 512
                                        ],
                                        start=(c == 0),
                                        stop=(c == DC - 1),
                                    )
                                )
                            ops.append(
                                lambda ps=ps, qk_sb=qk_sb, tt=tt:
                                nc.vector.tensor_copy(
                                    qk_sb[:, tt * 512 : (tt + 1) * 512], ps
                                )
                            )
                    return ops

                flush_mode = [False]

                def op_group_ops(t16, oc):
                    """Output-projection group for one [128 t, 512 oc] tile."""
                    holder = {}

                    def mm(s4):
                        def f():
                            if "ps" not in holder:
                                holder["ps"] = psa.tile(
                                    [P, 512], F32, tag="mm", bufs=2,
                                    name=f"ops{t16}_{oc}",
                                )
                            nc.tensor.matmul(
                                holder["ps"],
                                concat_sb[:, s4, t16 * P : (t16 + 1) * P],
                                wo_sb[:, s4, oc * 512 : (oc + 1) * 512],
                                start=(s4 == 0),
                                stop=(s4 == NPAIR - 1),
                            )
                        return f

                    def fin():
                        st_o = ap_.tile(
                            [P, 512], F16, tag="outst", bufs=6,
                            name=f"ost{t16}_{oc}",
                        )
                        if flush_mode[0] and (t16 + oc) % 2 == 0:
                            # ScalarE is exp-free during the final flush;
                            # alternate with DVE so the copies pipeline
                            nc.scalar.copy(st_o, holder["ps"])
                        else:
                            nc.vector.tensor_copy(st_o, holder["ps"])
                        if flush_mode[0]:
                            eng = nc.sync if oc == 0 else nc.scalar
                        else:
                            eng = nc.sync if oc == 0 else nc.gpsimd
                        eng.dma_start(
                            out=OUT[
                                t16 * P : (t16 + 1) * P,
                                oc * 512 : (oc + 1) * 512,
                            ],
                            in_=st_o,
                        )

                    return [mm(s4) for s4 in range(NPAIR)] + [fin]

                def drain_avs(pr, tt, avs):
                    """Free the 2-slot avs PSUM rotation fast (the next tt's
                    first AV matmul waits on it): one bulk copy per head,
                    then the whole normalize chain runs off the copy."""
                    cps, scp = [], []
                    for h2 in range(2):
                        cp = ap_.tile(
                            [K + 1, 512], F32, tag="avcp", bufs=4,
                            name=f"avcp{pr}_{tt}_{h2}",
                        )
                        if h2 == 1:
                            # ScalarE is exp-idle at pair/tail boundaries
                            nc.scalar.copy(cp, avs[h2])
                        else:
                            nc.vector.tensor_copy(cp, avs[h2])
                        cps.append(cp)
                    for h2 in range(2):
                        sp = ap_.tile(
                            [1, 512], F32, tag="scp", bufs=4,
                            name=f"scp{pr}_{tt}_{h2}",
                        )
                        nc.vector.tensor_copy(sp, cps[h2][K : K + 1, :])
                        scp.append(sp)
                    return cps, scp

                def direct_normalize(pr, tt, avs):
                    """Whole-tile normalize straight from the avs PSUM."""
                    for h in range(2):
                        cols = slice(tt * 512, (tt + 1) * 512)
                        sums = ap_.tile([1, 512], F32, tag="sums", bufs=6)
                        nc.vector.tensor_copy(sums, avs[h][K : K + 1, :])
                        recip = ap_.tile([1, 512], F32, tag="recip", bufs=6)
                        nc.vector.reciprocal_approx_fast(recip, sums)
                        bc_sb = ap_.tile([K, 512], F32, tag="bc_sb", bufs=6)
                        nc.gpsimd.partition_broadcast(bc_sb, recip)
                        if h == 0:
                            dst = concat_sb[0:K, pr, cols]
                        else:
                            dst = ap_.tile([K, 512], F16, tag="tmpb", bufs=6)
                        nc.vector.tensor_mul(dst, avs[h][0:K, :], bc_sb)
                        if h == 1:
                            nc.sync.dma_start(
                                out=concat_sb[K:P, pr, cols], in_=dst
                            )

                def normalize(pr, tt, cps, scp, cols_lo, width,
                              bounce_eng=None):
                    """1/sums for both heads over [cols_lo, cols_lo+width)."""
                    for h in range(2):
                        cols = slice(tt * 512 + cols_lo, tt * 512 + cols_lo + width)
                        psl = slice(cols_lo, cols_lo + width)
                        recip = ap_.tile([1, width], F32, tag="recip", bufs=6)
                        nc.vector.reciprocal_approx_fast(recip, scp[h][0:1, psl])
                        bc_sb = ap_.tile([K, width], F32, tag="bc_sb", bufs=6)
                        nc.gpsimd.partition_broadcast(bc_sb, recip)
                        if h == 0:
                            dst = concat_sb[0:K, pr, cols]
                        else:
                            dst = ap_.tile([K, width], F16, tag="tmpb", bufs=6)
                        nc.vector.tensor_mul(dst, cps[h][0:K, psl], bc_sb)
                        if h == 1:
                            # partition-shifted write via DMA bounce
                            (bounce_eng or nc.sync).dma_start(
                                out=concat_sb[K:P, pr, cols], in_=dst
                            )

                # upfront: warmup matmuls fill the X-DMA window and get the
                # PE HAM clock to 8/8 before real work; then V s0-3 and
                # pair 0's tt=0 Q/K — exactly what the first X block feeds.
                tiles0 = proj_weights(0, split=True)
                # X tail, emitted after the critical startup set so the
                # queue order matches arrival deadlines (tt1 needs
                # 512:1280 by ~25us, tt2 1024:1536 by ~35us, ...)
                for c in range(DC):
                    nc.sync.dma_start(
                        out=xt_sb[:, c, 512:1280], in_=xt_r[c][:, 512:1280]
                    )
                for c in range(DC):
                    (nc.scalar if c % 2 else nc.gpsimd).dma_start(
                        out=xt_sb[:, c, 1280:2048], in_=xt_r[c][:, 1280:2048]
                    )
                nc.gpsimd.dma_start(
                    out=wo_sb, in_=WO.rearrange("(s p) o -> p s o", p=P)
                )
                with tc.tile_pool(name="ps0", bufs=1, space="PSUM") as ps0:
                    warm_ps = ps0.tile([P, 512], F32, tag="warm", bufs=1)

                    def warm(n):
                        # dependency-free matmuls: keep the PE HAM activity
                        # window busy while real matmuls are DMA-paced
                        for _ in range(n):
                            nc.tensor.matmul(
                                warm_ps, warm_sb[:, 0:P], warm_sb,
                                start=True, stop=True,
                            )

                    warm(4)
                    for s in range(4):
                        for op in v_group_ops(s, pool=ps0, bufs=6):
                            op()
                    for op in proj_ops(0, tiles0, [0], pool=ps0, bufs=6):
                        op()
                psa_cm = tc.tile_pool(name="psa", bufs=1, space="PSUM")
                psa = psa_cm.__enter__()
                _, _, q_sb, k_sb = tiles0
                vqueue = [op for s in range(4, NS) for op in v_group_ops(s)]
                qk0queue = proj_ops(0, tiles0, [1, 2, 3])
                pending = []
                opqueue = []
                v_done = [0]
                qk0_done = [0]

                def pop_one(allow_op):
                    if vqueue:
                        vqueue.pop(0)()
                        v_done[0] += 1
                    elif qk0queue:
                        qk0queue.pop(0)()
                        qk0_done[0] += 1
                    elif pending:
                        pending.pop(0)()
                    elif allow_op and opqueue:
                        opqueue.pop(0)()

                for pr in range(NPAIR):
                    if pr < NPAIR - 1:
                        ntiles = proj_weights(pr + 1)
                        pending.extend(proj_ops(pr + 1, ntiles, range(NT)))
                    si_left = sum(4 * tt + 4 for tt in range(NT))

                    for tt in range(NT):
                        if pr == 0:
                            # V for this tt's key tiles and this tt's q/k
                            # chunks must be in flight before attention
                            need = (4 * tt) * 9
                            while v_done[0] < need and vqueue:
                                vqueue.pop(0)()
                                v_done[0] += 1
                            need_qk = 18 * tt
                            while qk0_done[0] < need_qk and qk0queue:
                                qk0queue.pop(0)()
                                qk0_done[0] += 1
                        avs = [
                            psa.tile(
                                [K + 1, 512], F32, tag="av", bufs=2,
                                name=f"av{pr}_{tt}_{h2}",
                            )
                            for h2 in range(2)
                        ]
                        n_s = 4 * tt + 4
                        for si in range(n_s):
                            # adaptive fill rate: spread the queued filler
                            # matmuls evenly over the pair's remaining steps
                            nq = (
                                len(vqueue) + len(qk0queue) + len(pending)
                                + len(opqueue)
                            )
                            pops = min(
                                6,
                                max(
                                    -(-nq // max(1, si_left)),
                                    3 if opqueue else 0,
                                ),
                            )
                            for _ in range(pops):
                                # op groups wait 2 steps so their concat
                                # DMA bounce never head-of-line blocks
                                pop_one(allow_op=(si >= 2))
                            si_left -= 1
                            # diagonal blocks: only cols >= 128*m can be valid
                            m = si - 4 * tt
                            off = max(m, 0) * P
                            nv = 512 - off
                            # both heads' score blocks in one 2-bank tile
                            st = psa.tile([P, 2, 512], F32, tag="stw", bufs=2)
                            ex = ap_.tile([P, 2, 512], F16, tag="exp", bufs=10)
                            for h in range(2):
                                lo, hi = h * K, (h + 1) * K
                                nc.tensor.matmul(
                                    st[:, h, 0:nv],
                                    k_sb[lo:hi, si * P : (si + 1) * P],
                                    q_sb[
                                        lo:hi,
                                        tt * 512 + off : (tt + 1) * 512,
                                    ],
                                    start=True,
                                    stop=True,
                                    tile_position=(lo, 0),
                                )
                            nc.scalar.activation(
                                ex[:, :, 0:nv], st[:, :, 0:nv],
                                mybir.ActivationFunctionType.Exp,
                                scale=0.125,
                            )
                            if m >= 0:  # mask both heads' leading triangles
                                nc.vector.tensor_mul(
                                    ex[:, :, 0:P],
                                    ex[:, :, 0:P],
                                    tri_sb.unsqueeze(1).broadcast_to(
                                        [P, 2, P]
                                    ),
                                )
                            for h in range(2):
                                slot = (2 * pr + h) * (K + 1)
                                nc.tensor.matmul(
                                    avs[h][:, off:512],
                                    v_sb[:, si, slot : slot + K + 1],
                                    ex[:, h, 0:nv],
                                    start=(si == 0),
                                    stop=(si == n_s - 1),
                                )
                        # Free the AV accumulators and normalize. Pair 3
                        # drains PSUM with bulk copies and normalizes in
                        # 128-col slices (bounces spread over two DMA
                        # queues) so each out-proj group unblocks early;
                        # pairs 0-2 normalize straight from PSUM.
                        if pr == NPAIR - 1:
                            cps, scp = drain_avs(pr, tt, avs)
                            for i16 in range(4):
                                normalize(
                                    pr, tt, cps, scp, i16 * P, P,
                                    bounce_eng=(
                                        nc.sync if i16 % 2 == 0 else nc.gpsimd
                                    ),
                                )
                                for oc in range(2):
                                    opqueue.extend(
                                        op_group_ops(4 * tt + i16, oc)
                                    )
                        elif tt == NT - 1:
                            # pair boundary: the next pair's first AV waits
                            # on this avs rotation — drain it fast so the
                            # boundary doesn't stall (and re-throttle HAM)
                            cps, scp = drain_avs(pr, tt, avs)
                            normalize(pr, tt, cps, scp, 0, 512)
                        else:
                            direct_normalize(pr, tt, avs)

                    # next pair's projections must be complete before its
                    # attention starts; flush whatever wasn't interleaved
                    while vqueue or qk0queue or pending:
                        pop_one(allow_op=False)
                    if pr < NPAIR - 1:
                        _, _, q_sb, k_sb = ntiles
                # bridge the final normalize chain: keep the PE busy so HAM
                # stays warm and the flush matmuls run at full clock
                warmf = psa.tile([P, 512], F32, tag="mm", bufs=2, name="warmf")
                for _ in range(10):
                    nc.tensor.matmul(
                        warmf, warm_sb[:, 0:P], warm_sb, start=True, stop=True
                    )
                flush_mode[0] = True
                while opqueue:
                    opqueue.pop(0)()
                psa_cm.__exit__(None, None, None)
    _fuse_score_ldweights(nc)
    nc.compile()
    return nc


def _fuse_score_ldweights(nc):
    """Merge each score pair's two 64-row LDWEIGHTS into one 128-row load.

    The post-Tile IR carries [Ldw(h0 64p), MM(0,0), Ldw(h1 64p), MM(64,0)]
    per key tile. With two LDWs the PE stalls ~100ns on each side of the
    pair (single background weight buffer). One 128-row LDW loads both
    heads' K slice at once; the row-tiled matmuls then address their own
    row groups of the already-loaded array.
    """
    fn = list(nc.m.functions)[0]
    fused = 0
    for blk in fn.blocks:
        insts = blk.instructions
        # pattern-match on the PE-engine subsequence: other engines'
        # instructions interleave freely in the block list
        pe = [
            (i, x)
            for i, x in enumerate(insts)
            if type(x).__name__ in ("InstLdweights", "InstMatmult")
        ]
        drop = []
        for k in range(len(pe) - 3):
            (_, a), (_, b), (ic, c), (_, d) = pe[k], pe[k + 1], pe[k + 2], pe[k + 3]
            if not (
                type(a).__name__ == "InstLdweights"
                and type(b).__name__ == "InstMatmult"
                and type(c).__name__ == "InstLdweights"
                and type(d).__name__ == "InstMatmult"
            ):
                continue
            if not (
                tuple(b.tile_size or ()) == (64, 128)
                and tuple(b.tile_position or ()) == (0, 0)
                and tuple(d.tile_size or ()) == (64, 128)
                and tuple(d.tile_position or ()) == (64, 0)
            ):
                continue
            apA, apC = a.ins[0], c.ins[0]
            pa, pc = list(apA.ap), list(apC.ap)
            if not (
                len(pa) == 2
                and pa[0][1] == 64
                and pc[0][1] == 64
                and pa[0][0] == pc[0][0]
                and pa[1] == pc[1]
                and apC.offset == apA.offset + 64 * pa[0][0]
                and c.sync_info is None
            ):
                continue
            apA.ap = [[pa[0][0], 128], pa[1]]
            if tuple(a.tile_size or ()) == (64, 128):
                a.tile_size = (128, 128)
            a.merge_dependencies_from(c)
            drop.append(ic)
            fused += 1
        for j in sorted(drop, reverse=True):
            del insts[j]
    assert fused > 0, "score LDW fusion matched nothing"


def shard_inputs(X, Wq, Wk, Wv, Wo):
    """Host-side shard prep: core c handles batch c//2, head group c%2."""
    in_maps = []
    for c in range(8):
        b, g = c // 2, c % 2
        heads = range(g * HG, (g + 1) * HG)
        wq = np.stack(
            [
                np.concatenate([Wq[g * HG + 2 * p], Wq[g * HG + 2 * p + 1]], axis=1)
                for p in range(NPAIR)
            ]
        )
        wk = np.stack(
            [
                np.concatenate([Wk[g * HG + 2 * p], Wk[g * HG + 2 * p + 1]], axis=1)
                for p in range(NPAIR)
            ]
        )
        wv = np.concatenate([Wv[h] for h in heads], axis=1)
        wo = Wo[:, g * 512 : (g + 1) * 512].T
        in_maps.append(
            {
                "xt": np.ascontiguousarray(X[b].T).astype(np.float16),
                "wq": np.ascontiguousarray(wq).astype(np.float16),
                "wk": np.ascontiguousarray(wk).astype(np.float16),
                "wv": np.ascontiguousarray(wv).astype(np.float16),
                "wo": np.ascontiguousarray(wo).astype(np.float16),
            }
        )
    return in_maps


_MODULE = None


def _get_module():
    global _MODULE
    if _MODULE is None:
        _MODULE = build_module()
    return _MODULE


def kernel(X, Wq, Wk, Wv, Wo, bo, _want_results=None):
    from concourse.bass_utils import run_bass_kernel_spmd

    nc = _get_module()
    in_maps = shard_inputs(
        np.asarray(X), np.asarray(Wq), np.asarray(Wk), np.asarray(Wv), np.asarray(Wo)
    )
    res = run_bass_kernel_spmd(nc, in_maps, core_ids=list(range(8)))
    if _want_results is not None:
        _want_results.append(res)
    out = np.empty((B, T, H * K), dtype=np.float32)
    bo = np.asarray(bo, dtype=np.float32)
    for b in range(B):
        out[b] = (
            res.results[2 * b]["out"].astype(np.float32)
            + res.results[2 * b + 1]["out"].astype(np.float32)
            + bo
        )
    return out


# revision 46
# speedup vs baseline: 1.0048x; 1.0048x over previous
"""Multi-head causal self-attention on 8 Trainium2 NeuronCores.

Problem: X[4,2048,1024], per-head Wq/Wk/Wv[16,1024,64], Wo[1024,1024], bo[1024].
    out = OutProj(concat_heads(softmax_causal(Q K^T / 8) V))

Sharding: 8 cores = 4 batches x 2 head-groups (8 heads each). Each core
computes its batch's attention for its 8 heads plus the partial output
projection over its 512 concat features; host sums the two partials per
batch and adds the bias.

Per-core kernel (matmul operands in fp16 — 1 cycle/row on TensorE and
fp32 PSUM accumulation; softmax runs in the transposed
"feature-on-partition" space so its reduction lands on the free dim):
  qT/kT per head-pair  [128, T]  = Wpair^T  x  X^T
  v    per s-tile      [128, 8*65] = X^T^T  x  Wv_all (65th col set to 1)
  ST block [s=128, t=512] = kT_slice^T @ qT_slice   (row-packed head pairs)
  expST = exp(ST/8) (ScalarE), causal-masked via tri multiply
  avT [65, 512] += [V|1]^T @ expST   -> rows 0:64 = (A@V)^T, row 64 = sums
  normalize via 1/sums broadcast and write concatT
  partial = concatT^T @ WoST  (accumulated over 4 feature chunks)

Scheduling: everything is emitted in DMA-arrival order. X streams in
T-block order; pair 0's tt>0 projections are deferred into its attention
loop (need-forced per query tile) so the first scores fire as soon as
the first X block lands. Warmup matmuls fill the DMA window and keep the
PE HAM clock at 8/8. The output projection is interleaved into pair 3's
attention as PE filler; pair 3 normalizes in 128-column slices so each
out-proj group unblocks as early as possible.
"""

import os
import sys

for _p in ("/opt/trn_rl_repo", "/root/.axon_site/_ro/trn_rl_repo"):
    if os.path.isdir(_p) and _p not in sys.path:
        sys.path.append(_p)

import numpy as np

import concourse.mybir as mybir
import concourse.tile as tile
from concourse import bacc

B, T, D, H, K = 4, 2048, 1024, 16, 64
HG = 8          # heads per core
NPAIR = 4       # head pairs per core
P = 128
DC = D // P     # 8 contraction chunks for the projections
NS = T // P     # 16 key tiles
NT = T // 512   # 4 query tiles of 512
F32 = mybir.dt.float32
F16 = mybir.dt.float16


def build_module():
    nc = bacc.Bacc("TRN2")
    XT = nc.dram_tensor("xt", [D, T], F16, kind="ExternalInput").ap()
    WQ = nc.dram_tensor("wq", [NPAIR, D, P], F16, kind="ExternalInput").ap()
    WK = nc.dram_tensor("wk", [NPAIR, D, P], F16, kind="ExternalInput").ap()
    WV = nc.dram_tensor("wv", [D, HG * K], F16, kind="ExternalInput").ap()
    WO = nc.dram_tensor("wo", [HG * K, D], F16, kind="ExternalInput").ap()
    OUT = nc.dram_tensor("out", [T, D], F16, kind="ExternalOutput").ap()

    with tile.TileContext(nc) as tc:
        with tc.tile_pool(name="persist", bufs=1) as pp:
            xt_sb = pp.tile([P, DC, T], F16)            # X^T, 32 KB/partition
            v_sb = pp.tile([P, NS, HG * (K + 1)], F16)  # V + ones col per head
            concat_sb = pp.tile([P, NPAIR, T], F16)     # concat(heads)^T
            tri_sb = pp.tile([P, P], F16)   # causal triangle: 1 where x >= p
            warm_sb = pp.tile([P, 512], F16)

            # X^T in T-block order: the first 512 token-cols of every chunk
            # land first so V/Q/K tile 0 start as soon as possible. Each
            # DMA queue only sustains ~110-220 GB/s, so the critical
            # startup set (first X block + Wv + pair-0 Wq/Wk) is spread
            # over all three trigger queues; the X tail and Wo follow in
            # arrival-deadline order.
            xt_r = XT.rearrange("(c p) t -> c p t", p=P)
            wo_sb = pp.tile([P, NPAIR, D], F16)
            for c in range(DC):
                nc.sync.dma_start(out=xt_sb[:, c, 0:512], in_=xt_r[c][:, 0:512])
            nc.vector.memset(warm_sb, 0.0)
            nc.vector.memset(tri_sb, 1.0)
            nc.gpsimd.affine_select(
                out=tri_sb,
                in_=tri_sb,
                compare_op=mybir.AluOpType.is_ge,
                fill=0.0,
                base=0,
                channel_multiplier=-1,
                pattern=[[1, P]],
            )
            # ones column (index 64 of each head's 65-wide slot)
            v_slots = v_sb.rearrange("p s (h x) -> p s h x", x=K + 1)
            nc.vector.memset(v_slots[:, :, :, K : K + 1], 1.0)

            # ---- V + Q/K projections + attention, software-pipelined ----
            # Projection matmuls (V tail, pair-0 tt>0, the NEXT pair's Q/K)
            # and pair-3's output projection are interleaved into the
            # attention loop so the PE fills the stalls where it would
            # otherwise wait on ScalarE's exp.
            with (
                tc.tile_pool(name="wvp", bufs=1) as wvp,
                tc.tile_pool(name="attn", bufs=1) as ap_,
            ):
                psa = None  # assigned after the startup PSUM pool closes
                wv_sb = wvp.tile([P, DC, HG * K], F16)
                wv_r = WV.rearrange("(c p) n -> c p n", p=P)
                for c in range(DC):
                    (nc.scalar if c < 4 else nc.gpsimd).dma_start(
                        out=wv_sb[:, c, :], in_=wv_r[c]
                    )

                def v_group_ops(s, pool=None, bufs=2):
                    ps = (pool or psa).tile(
                        [P, HG * K], F32, tag="mm", bufs=bufs, name=f"vps{s}"
                    )
                    ops = [
                        (
                            lambda c=c, ps=ps, s=s: nc.tensor.matmul(
                                ps,
                                xt_sb[:, c, s * P : (s + 1) * P],
                                wv_sb[:, c, :],
                                start=(c == 0),
                                stop=(c == DC - 1),
                            )
                        )
                        for c in range(DC)
                    ]
                    ops.append(
                        lambda ps=ps, s=s: nc.vector.tensor_copy(
                            v_slots[:, s, :, 0:K],
                            ps.rearrange("p (h k) -> p h k", k=K),
                        )
                    )
                    return ops

                def proj_weights(pr, split=False):
                    """DMA the pair's Wq/Wk; alloc the q/k destinations."""
                    wq_sb = ap_.tile(
                        [P, DC, P], F16, tag="wq", bufs=2, name=f"wq{pr}"
                    )
                    wk_sb = ap_.tile(
                        [P, DC, P], F16, tag="wk", bufs=2, name=f"wk{pr}"
                    )
                    nc.scalar.dma_start(
                        out=wq_sb, in_=WQ[pr].rearrange("(c p) m -> p c m", p=P)
                    )
                    (nc.gpsimd if split else nc.scalar).dma_start(
                        out=wk_sb, in_=WK[pr].rearrange("(c p) m -> p c m", p=P)
                    )
                    q_sb = ap_.tile([P, T], F16, tag="q", bufs=2, name=f"q{pr}")
                    k_sb = ap_.tile([P, T], F16, tag="k", bufs=2, name=f"k{pr}")
                    return wq_sb, wk_sb, q_sb, k_sb

                def proj_ops(pr, tiles, tts, pool=None, bufs=2):
                    """Projection matmuls in tt-major (DMA-arrival) order."""
                    wq_sb, wk_sb, q_sb, k_sb = tiles
                    ops = []
                    for tt in tts:
                        for w_sb, qk_sb, nm in (
                            (wq_sb, q_sb, "q"),
                            (wk_sb, k_sb, "k"),
                        ):
                            ps = (pool or psa).tile(
                                [P, 512], F32, tag="mm", bufs=bufs,
                                name=f"{nm}ps{pr}_{tt}",
                            )
                            for c in range(DC):
                                ops.append(
                                    lambda ps=ps, w_sb=w_sb, c=c, tt=tt:
                                    nc.tensor.matmul(
                                        ps,
                                        w_sb[:, c, :],
                                        xt_sb[
                                            :, c, tt * 512 : (tt + 1) * 512
                                        ],
                                        start=(c == 0),
                                        stop=(c == DC - 1),
                                    )
                                )
                            ops.append(
                                lambda ps=ps, qk_sb=qk_sb, tt=tt:
                                nc.vector.tensor_copy(
                                    qk_sb[:, tt * 512 : (tt + 1) * 512], ps
                                )
                            )
                    return ops

                flush_mode = [False]

                def op_group_ops(t16, oc):
                    """Output-projection group for one [128 t, 512 oc] tile."""
                    holder = {}

                    def mm(s4):
                        def f():
                            if "ps" not in holder:
                                holder["ps"] = psa.tile(
                                    [P, 512], F32, tag="mm", bufs=2,
                                    name=f"ops{t16}_{oc}",
                                )
                            nc.tensor.matmul(
                                holder["ps"],
                                concat_sb[:, s4, t16 * P : (t16 + 1) * P],
                                wo_sb[:, s4, oc * 512 : (oc + 1) * 512],
                                start=(s4 == 0),
                                stop=(s4 == NPAIR - 1),
                            )
                        return f

                    def fin():
                        st_o = ap_.tile(
                            [P, 512], F16, tag="outst", bufs=6,
                            name=f"ost{t16}_{oc}",
                        )
                        if flush_mode[0] and (t16 + oc) % 2 == 0:
                            # ScalarE is exp-free during the final flush;
                            # alternate with DVE so the copies pipeline
                            nc.scalar.copy(st_o, holder["ps"])
                        else:
                            nc.vector.tensor_copy(st_o, holder["ps"])
                        if flush_mode[0]:
                            eng = nc.sync if oc == 0 else nc.scalar
                        else:
                            eng = nc.sync if oc == 0 else nc.gpsimd
                        eng.dma_start(
                            out=OUT[
                                t16 * P : (t16 + 1) * P,
                                oc * 512 : (oc + 1) * 512,
                            ],
                            in_=st_o,
                        )

                    return [mm(s4) for s4 in range(NPAIR)] + [fin]

                def drain_avs(pr, tt, avs):
                    """Free the 2-slot avs PSUM rotation fast (the next tt's
                    first AV matmul waits on it): one bulk copy per head,
                    then the whole normalize chain runs off the copy."""
                    cps, scp = [], []
                    for h2 in range(2):
                        cp = ap_.tile(
                            [K + 1, 512], F32, tag="avcp", bufs=4,
                            name=f"avcp{pr}_{tt}_{h2}",
                        )
                        if h2 == 1:
                            # ScalarE is exp-idle at pair/tail boundaries
                            nc.scalar.copy(cp, avs[h2])
                        else:
                            nc.vector.tensor_copy(cp, avs[h2])
                        cps.append(cp)
                    for h2 in range(2):
                        sp = ap_.tile(
                            [1, 512], F32, tag="scp", bufs=4,
                            name=f"scp{pr}_{tt}_{h2}",
                        )
                        nc.vector.tensor_copy(sp, cps[h2][K : K + 1, :])
                        scp.append(sp)
                    return cps, scp

                def direct_normalize(pr, tt, avs):
                    """Whole-tile normalize straight from the avs PSUM."""
                    for h in range(2):
                        cols = slice(tt * 512, (tt + 1) * 512)
                        sums = ap_.tile([1, 512], F32, tag="sums", bufs=6)
                        nc.vector.tensor_copy(sums, avs[h][K : K + 1, :])
                        recip = ap_.tile([1, 512], F32, tag="recip", bufs=6)
                        nc.vector.reciprocal_approx_fast(recip, sums)
                        bc_sb = ap_.tile([K, 512], F32, tag="bc_sb", bufs=6)
                        nc.gpsimd.partition_broadcast(bc_sb, recip)
                        if h == 0:
                            dst = concat_sb[0:K, pr, cols]
                        else:
                            dst = ap_.tile([K, 512], F16, tag="tmpb", bufs=6)
                        nc.vector.tensor_mul(dst, avs[h][0:K, :], bc_sb)
                        if h == 1:
                            nc.sync.dma_start(
                                out=concat_sb[K:P, pr, cols], in_=dst
                            )

                def normalize(pr, tt, cps, scp, cols_lo, width,
                              bounce_eng=None):
                    """1/sums for both heads over [cols_lo, cols_lo+width)."""
                    for h in range(2):
                        cols = slice(tt * 512 + cols_lo, tt * 512 + cols_lo + width)
                        psl = slice(cols_lo, cols_lo + width)
                        recip = ap_.tile([1, width], F32, tag="recip", bufs=6)
                        nc.vector.reciprocal_approx_fast(recip, scp[h][0:1, psl])
                        bc_sb = ap_.tile([K, width], F32, tag="bc_sb", bufs=6)
                        nc.gpsimd.partition_broadcast(bc_sb, recip)
                        if h == 0:
                            dst = concat_sb[0:K, pr, cols]
                        else:
                            dst = ap_.tile([K, width], F16, tag="tmpb", bufs=6)
                        nc.vector.tensor_mul(dst, cps[h][0:K, psl], bc_sb)
                        if h == 1:
                            # partition-shifted write via DMA bounce
                            (bounce_eng or nc.sync).dma_start(
                                out=concat_sb[K:P, pr, cols], in_=dst
                            )

                # upfront: warmup matmuls fill the X-DMA window and get the
                # PE HAM clock to 8/8 before real work; then V s0-3 and
                # pair 0's tt=0 Q/K — exactly what the first X block feeds.
                tiles0 = proj_weights(0, split=True)
                # X tail, emitted after the critical startup set so the
                # queue order matches arrival deadlines (tt1 needs
                # 512:1280 by ~25us, tt2 1024:1536 by ~35us, ...)
                for c in range(DC):
                    nc.sync.dma_start(
                        out=xt_sb[:, c, 512:1280], in_=xt_r[c][:, 512:1280]
                    )
                for c in range(DC):
                    (nc.scalar if c % 2 else nc.gpsimd).dma_start(
                        out=xt_sb[:, c, 1280:2048], in_=xt_r[c][:, 1280:2048]
                    )
                nc.gpsimd.dma_start(
                    out=wo_sb, in_=WO.rearrange("(s p) o -> p s o", p=P)
                )
                with tc.tile_pool(name="ps0", bufs=1, space="PSUM") as ps0:
                    warm_ps = ps0.tile([P, 512], F32, tag="warm", bufs=1)

                    def warm(n):
                        # dependency-free matmuls: keep the PE HAM activity
                        # window busy while real matmuls are DMA-paced
                        for _ in range(n):
                            nc.tensor.matmul(
                                warm_ps, warm_sb[:, 0:P], warm_sb,
                                start=True, stop=True,
                            )

                    warm(4)
                    for s in range(4):
                        for op in v_group_ops(s, pool=ps0, bufs=6):
                            op()
                    for op in proj_ops(0, tiles0, [0], pool=ps0, bufs=6):
                        op()
                psa_cm = tc.tile_pool(name="psa", bufs=1, space="PSUM")
                psa = psa_cm.__enter__()
                _, _, q_sb, k_sb = tiles0
                vqueue = [op for s in range(4, NS) for op in v_group_ops(s)]
                qk0queue = proj_ops(0, tiles0, [1, 2, 3])
                pending = []
                opqueue = []
                v_done = [0]
                qk0_done = [0]

                def pop_one(allow_op):
                    if vqueue:
                        vqueue.pop(0)()
                        v_done[0] += 1
                    elif qk0queue:
                        qk0queue.pop(0)()
                        qk0_done[0] += 1
                    elif pending:
                        pending.pop(0)()
                    elif allow_op and opqueue:
                        opqueue.pop(0)()

                for pr in range(NPAIR):
                    if pr < NPAIR - 1:
                        ntiles = proj_weights(pr + 1)
                        pending.extend(proj_ops(pr + 1, ntiles, range(NT)))
                    si_left = sum(4 * tt + 4 for tt in range(NT))

                    for tt in range(NT):
                        if pr == 0:
                            # V for this tt's key tiles and this tt's q/k
                            # chunks must be in flight before attention
                            need = (4 * tt) * 9
                            while v_done[0] < need and vqueue:
                                vqueue.pop(0)()
                                v_done[0] += 1
                            need_qk = 18 * tt
                            while qk0_done[0] < need_qk and qk0queue:
                                qk0queue.pop(0)()
                                qk0_done[0] += 1
                        avs = [
                            psa.tile(
                                [K + 1, 512], F32, tag="av", bufs=2,
                                name=f"av{pr}_{tt}_{h2}",
                            )
                            for h2 in range(2)
                        ]
                        n_s = 4 * tt + 4
                        for si in range(n_s):
                            # adaptive fill rate: spread the queued filler
                            # matmuls evenly over the pair's remaining steps
                            nq = (
                                len(vqueue) + len(qk0queue) + len(pending)
                                + len(opqueue)
                            )
                            pops = min(
                                6,
                                max(
                                    -(-nq // max(1, si_left)),
                                    3 if opqueue else 0,
                                ),
                            )
                            for _ in range(pops):
                                # op groups wait 2 steps so their concat
                                # DMA bounce never head-of-line blocks
                                pop_one(allow_op=(si >= 2))
                            si_left -= 1
                            # diagonal blocks: only cols >= 128*m can be valid
                            m = si - 4 * tt
                            off = max(m, 0) * P
                            nv = 512 - off
                            # both heads' score blocks in one 2-bank tile
                            st = psa.tile([P, 2, 512], F32, tag="stw", bufs=2)
                            ex = ap_.tile([P, 2, 512], F16, tag="exp", bufs=10)
                            for h in range(2):
                                lo, hi = h * K, (h + 1) * K
                                nc.tensor.matmul(
                                    st[:, h, 0:nv],
                                    k_sb[lo:hi, si * P : (si + 1) * P],
                                    q_sb[
                                        lo:hi,
                                        tt * 512 + off : (tt + 1) * 512,
                                    ],
                                    start=True,
                                    stop=True,
                                    tile_position=(lo, 0),
                                )
                            nc.scalar.activation(
                                ex[:, :, 0:nv], st[:, :, 0:nv],
                                mybir.ActivationFunctionType.Exp,
                                scale=0.125,
                            )
                            if m >= 0:  # mask both heads' leading triangles
                                nc.vector.tensor_mul(
                                    ex[:, :, 0:P],
                                    ex[:, :, 0:P],
                                    tri_sb.unsqueeze(1).broadcast_to(
                                        [P, 2, P]
                                    ),
                                )
                            for h in range(2):
                                slot = (2 * pr + h) * (K + 1)
                                nc.tensor.matmul(
                                    avs[h][:, off:512],
                                    v_sb[:, si, slot : slot + K + 1],
                                    ex[:, h, 0:nv],
                                    start=(si == 0),
                                    stop=(si == n_s - 1),
                                )
                        # Free the AV accumulators and normalize. Pair 3
                        # drains PSUM with bulk copies and normalizes in
                        # 128-col slices (bounces spread over two DMA
                        # queues) so each out-proj group unblocks early;
                        # pairs 0-2 normalize straight from PSUM.
                        if pr == NPAIR - 1:
                            cps, scp = drain_avs(pr, tt, avs)
                            for i16 in range(4):
                                normalize(
                                    pr, tt, cps, scp, i16 * P, P,
                                    bounce_eng=(
                                        nc.sync if i16 % 2 == 0 else nc.gpsimd
                                    ),
                                )
                                for oc in range(2):
                                    opqueue.extend(
                                        op_group_ops(4 * tt + i16, oc)
                                    )
                        else:
                            direct_normalize(pr, tt, avs)

                    # next pair's projections must be complete before its
                    # attention starts; flush whatever wasn't interleaved
                    while vqueue or qk0queue or pending:
                        pop_one(allow_op=False)
                    if pr < NPAIR - 1:
                        _, _, q_sb, k_sb = ntiles
                # bridge the final normalize chain: keep the PE busy so HAM
                # stays warm and the flush matmuls run at full clock
                warmf = psa.tile([P, 512], F32, tag="mm", bufs=2, name="warmf")
                for _ in range(10):
                    nc.tensor.matmul(
                        warmf, warm_sb[:, 0:P], warm_sb, start=True, stop=True
                    )
                flush_mode[0] = True
                while opqueue:
                    opqueue.pop(0)()
                psa_cm.__exit__(None, None, None)
    _fuse_score_ldweights(nc)
    nc.compile()
    return nc


def _fuse_score_ldweights(nc):
    """Merge each score pair's two 64-row LDWEIGHTS into one 128-row load.

    The post-Tile IR carries [Ldw(h0 64p), MM(0,0), Ldw(h1 64p), MM(64,0)]
    per key tile. With two LDWs the PE stalls ~100ns on each side of the
    pair (single background weight buffer). One 128-row LDW loads both
    heads' K slice at once; the row-tiled matmuls then address their own
    row groups of the already-loaded array.
    """
    fn = list(nc.m.functions)[0]
    fused = 0
    for blk in fn.blocks:
        insts = blk.instructions
        # pattern-match on the PE-engine subsequence: other engines'
        # instructions interleave freely in the block list
        pe = [
            (i, x)
            for i, x in enumerate(insts)
            if type(x).__name__ in ("InstLdweights", "InstMatmult")
        ]
        drop = []
        for k in range(len(pe) - 3):
            (_, a), (_, b), (ic, c), (_, d) = pe[k], pe[k + 1], pe[k + 2], pe[k + 3]
            if not (
                type(a).__name__ == "InstLdweights"
                and type(b).__name__ == "InstMatmult"
                and type(c).__name__ == "InstLdweights"
                and type(d).__name__ == "InstMatmult"
            ):
                continue
            if not (
                tuple(b.tile_size or ()) == (64, 128)
                and tuple(b.tile_position or ()) == (0, 0)
                and tuple(d.tile_size or ()) == (64, 128)
                and tuple(d.tile_position or ()) == (64, 0)
            ):
                continue
            apA, apC = a.ins[0], c.ins[0]
            pa, pc = list(apA.ap), list(apC.ap)
            if not (
                len(pa) == 2
                and pa[0][1] == 64
                and pc[0][1] == 64
                and pa[0][0] == pc[0][0]
                and pa[1] == pc[1]
                and apC.offset == apA.offset + 64 * pa[0][0]
                and c.sync_info is None
            ):
                continue
            apA.ap = [[pa[0][0], 128], pa[1]]
            if tuple(a.tile_size or ()) == (64, 128):
                a.tile_size = (128, 128)
            a.merge_dependencies_from(c)
            drop.append(ic)
            fused += 1
        for j in sorted(drop, reverse=True):
            del insts[j]
    assert fused > 0, "score LDW fusion matched nothing"


def shard_inputs(X, Wq, Wk, Wv, Wo):
    """Host-side shard prep: core c handles batch c//2, head group c%2."""
    in_maps = []
    for c in range(8):
        b, g = c // 2, c % 2
        heads = range(g * HG, (g + 1) * HG)
        wq = np.stack(
            [
                np.concatenate([Wq[g * HG + 2 * p], Wq[g * HG + 2 * p + 1]], axis=1)
                for p in range(NPAIR)
            ]
        )
        wk = np.stack(
            [
                np.concatenate([Wk[g * HG + 2 * p], Wk[g * HG + 2 * p + 1]], axis=1)
                for p in range(NPAIR)
            ]
        )
        wv = np.concatenate([Wv[h] for h in heads], axis=1)
        wo = Wo[:, g * 512 : (g + 1) * 512].T
        in_maps.append(
            {
                "xt": np.ascontiguousarray(X[b].T).astype(np.float16),
                "wq": np.ascontiguousarray(wq).astype(np.float16),
                "wk": np.ascontiguousarray(wk).astype(np.float16),
                "wv": np.ascontiguousarray(wv).astype(np.float16),
                "wo": np.ascontiguousarray(wo).astype(np.float16),
            }
        )
    return in_maps


_MODULE = None


def _get_module():
    global _MODULE
    if _MODULE is None:
        _MODULE = build_module()
    return _MODULE


def kernel(X, Wq, Wk, Wv, Wo, bo, _want_results=None):
    from concourse.bass_utils import run_bass_kernel_spmd

    nc = _get_module()
    in_maps = shard_inputs(
        np.asarray(X), np.asarray(Wq), np.asarray(Wk), np.asarray(Wv), np.asarray(Wo)
    )
    res = run_bass_kernel_spmd(nc, in_maps, core_ids=list(range(8)))
    if _want_results is not None:
        _want_results.append(res)
    out = np.empty((B, T, H * K), dtype=np.float32)
    bo = np.asarray(bo, dtype=np.float32)
    for b in range(B):
        out[b] = (
            res.results[2 * b]["out"].astype(np.float32)
            + res.results[2 * b + 1]["out"].astype(np.float32)
            + bo
        )
    return out
